# revision 13
# baseline (speedup 1.0000x reference)
"""YOLO-detect head (1x1 conv + box decode) on 8 Trainium2 NeuronCores.

Data-parallel over batch: core b processes batch element b.

Per core, per level l (C channels, HW = ny*nx positions):
  p[hw, o] = sum_c x[c, hw] * w[o, c]      (o = a*89 + ch, a anchor, ch channel)
computed on the tensor engine as out = lhsT.T @ rhs with
  lhsT = x chunk  [K channels, M<=128 hw]   (stationary)
  rhs  = w.T chunk [K channels, N=267]      (moving)
so the PSUM result is already [hw, 267] - no on-chip transpose.
Levels 0-1 run in fp8(e4m3) DoubleRow mode: K=256 contracted per instruction
(w0/w1 host-prescaled by 16 to clear e4m3's subnormal range; compensated by
the activation input scale 1/16). Level 2 stays fp16 (its biggest anchors
amplify exp() error, so it keeps the precision).

Decode: the ACT engine uses the sigmoid table DIRECTLY (one fp16 sigmoid of
all 89 cols per psum group; the ONLY psum reader, so psum groups free as
early as possible). The exp table is never needed -> one table load total.
  wh:  u = 1-s (DVE), u <- 1/u (fast approx reciprocal), q = s*u = exp(p),
       wh = q*anchor
  xy:  xy = s*stride + grid*stride
These fixups read the RESIDENT sigmoid tile (not psum) and therefore run
per ~8-12-tile store chunk, off the psum critical path.

Schedule: levels are interleaved into one global group stream ordered by
input-arrival time (L0 first, L1 woven in from slot 6, L2 near the end) so
the scalar engine's sigmoid stream - the throughput floor of this kernel -
never starves while matmuls for later levels fill PE idle slots.

DMA: x0 pieces load on nc.sync's HWDGE ring; everything else loads on
nc.vector's ring in parallel (descriptor generation is ~0.65us each and
serializes per-engine). Early big store chunks go via nc.gpsimd (SWDGE);
final small chunks via nc.sync so the drain tail is short.
"""

import numpy as np
import ml_dtypes

import concourse.bacc as bacc
import concourse.mybir as mybir
import concourse.tile as tile
from concourse.bass_utils import run_bass_kernel_spmd

F32 = mybir.dt.float32
F16 = mybir.dt.float16
F8 = mybir.dt.float8e4
AF = mybir.ActivationFunctionType
ALU = mybir.AluOpType
DR = mybir.MatmulPerfMode.DoubleRow
NP_F8 = ml_dtypes.float8_e4m3fn

NCORES = 8
NA = 3          # anchors per level
NO = 89         # channels per anchor (80 classes + 5 + 4)
NCOL = NA * NO  # 267
GROUP = 4       # full 128-row hw tiles per PSUM group (4 banks; 2 bufs = all 8)
WSCALE = 16.0   # host pre-scale on fp8 weights (subnormal avoidance)

LEVELS = [
    dict(C=256,  W=80, HW=6400, stride=8.0, fp8=True,
         anchors=((10.0, 13.0), (16.0, 30.0), (33.0, 23.0))),
    dict(C=512,  W=40, HW=1600, stride=16.0, fp8=True,
         anchors=((30.0, 61.0), (62.0, 45.0), (59.0, 119.0))),
    dict(C=1024, W=20, HW=400,  stride=32.0, fp8=False,
         anchors=((116.0, 90.0), (156.0, 198.0), (373.0, 326.0))),
]
NT = [(L["HW"] + 127) // 128 for L in LEVELS]   # 50, 13, 4
NTSUM = sum(NT)                                  # 67
LOFF = [sum(NT[:l]) for l in range(3)]

# (t0, ntiles, rows) per level
GROUPS = []
for _l, _L in enumerate(LEVELS):
    full, rem = divmod(_L["HW"], 128)
    g = []
    t0 = 0
    while t0 < full:
        n = min(GROUP, full - t0)
        g.append((t0, n, 128))
        t0 += n
    if rem:
        g.append((full, 1, rem))
    GROUPS.append(g)

# global schedule: (level, group_idx); L1 woven in from slot 6, L2 last
SCHED = []
_q0 = [(0, i) for i in range(len(GROUPS[0]))]
_q1 = [(1, i) for i in range(len(GROUPS[1]))]
_q2 = [(2, i) for i in range(len(GROUPS[2]))]
SCHED += _q0[:6]
_rest0 = _q0[6:]
while _q1:
    SCHED.append(_q1.pop(0))
    if _rest0:
        SCHED.append(_rest0.pop(0))
SCHED.append(_q2[0])
SCHED += _rest0
SCHED.append(_q2[1])

# store chunks (t0, ntiles, engine): early big ones on gpsimd SWDGE, the
# final ones on sync whose load queue has drained by then
CHUNKS = [
    [(0, 12, "g"), (12, 12, "g"), (24, 12, "g"), (36, 8, "g"), (44, 6, "s")],
    [(0, 8, "g"), (8, 5, "s")],
    [(0, 4, "s")],
]
CHMAX = 12


def _build_program(use_bias: bool):
    # Bacc (not raw Bass): its compile() runs move_matmul_waits_to_ldweights +
    # generate_event_semaphores, without which walrus rejects instructions
    # that carry more than one semaphore wait.
    nc = bacc.Bacc("TRN2", target_bir_lowering=False, debug=False)

    dram = {}
    for l, L in enumerate(LEVELS):
        KD = L["C"] // 256
        if L["fp8"]:
            dram[f"x{l}"] = nc.dram_tensor(f"x{l}", (128, KD, 2, L["HW"]), F8,
                                           kind="ExternalInput").ap()
            dram[f"wt{l}"] = nc.dram_tensor(f"wt{l}", (128, KD, 2, NCOL), F8,
                                            kind="ExternalInput").ap()
        else:
            KC = L["C"] // 128
            dram[f"x{l}"] = nc.dram_tensor(f"x{l}", (128, KC * L["HW"]), F16,
                                           kind="ExternalInput").ap()
            dram[f"wt{l}"] = nc.dram_tensor(f"wt{l}", (128, KC * NCOL), F16,
                                            kind="ExternalInput").ap()
        nt = NT[l]
        dram[f"y89_{l}"] = nc.dram_tensor(f"y89_{l}", (128, NA, nt, NO), F16,
                                          kind="ExternalOutput").ap()
        dram[f"y4_{l}"] = nc.dram_tensor(f"y4_{l}", (128, NA, nt, 4), F16,
                                         kind="ExternalOutput").ap()
        if use_bias:
            dram[f"b{l}"] = nc.dram_tensor(f"b{l}", (1, NCOL), F32,
                                           kind="ExternalInput").ap()
    # gat[p, t, a, 0:2] = grid*stride for hw row t*128+p (replicated over a)
    # gat[p, t, a, 2:4] = anchor wh (replicated over t)
    dram["gat"] = nc.dram_tensor("gat", (128, NTSUM, NA, 4), F16,
                                 kind="ExternalInput").ap()

    with tile.TileContext(nc) as tc:
        with tc.tile_pool(name="consts", bufs=1) as cpool, \
             tc.tile_pool(name="xbuf", bufs=1) as xpool, \
             tc.tile_pool(name="obuf", bufs=1) as opool, \
             tc.tile_pool(name="scr", bufs=2) as spool, \
             tc.tile_pool(name="ps", bufs=2, space="PSUM") as pspool:

            ones_t = None
            if use_bias:
                ones_t = cpool.tile([1, 128], F16, tag="ones", name="ones")
                nc.vector.memset(ones_t[:, :], 1.0)

            # ---- Phase A: loads on two parallel HWDGE rings ----
            lvl = {}
            # sync ring: x0 only (paced for the level-0 stream start)
            wt0_t = cpool.tile([128, 1, 2, NCOL], F8, tag="wt0", name="wt0sb")
            nc.sync.dma_start(out=wt0_t[:, :, :, :], in_=dram["wt0"][:, :, :, :])
            xk0 = xpool.tile([128, 1, 2, LEVELS[0]["HW"]], F8, tag="x0",
                             name="xk0")
            for (c0, c1) in ((0, 1024), (1024, 3072), (3072, LEVELS[0]["HW"])):
                nc.sync.dma_start(out=xk0[:, :, :, c0:c1],
                                  in_=dram["x0"][:, :, :, c0:c1])
            # scalar ring (desc-gen finishes before its sigmoid stream starts)
            wt1_t = cpool.tile([128, 2, 2, NCOL], F8, tag="wt1", name="wt1sb")
            nc.scalar.dma_start(out=wt1_t[:, :, :, :],
                                in_=dram["wt1"][:, :, :, :])
            xk1 = xpool.tile([128, 2, 2, LEVELS[1]["HW"]], F8, tag="x1",
                             name="xk1")
            nc.scalar.dma_start(out=xk1[:, :, :, :], in_=dram["x1"][:, :, :, :])
            gat_t = cpool.tile([128, NTSUM, NA, 4], F16, tag="gat",
                               name="gatsb")
            nc.scalar.dma_start(out=gat_t[:, :, :, :], in_=dram["gat"][:, :, :, :])
            # gpsimd SWDGE ring: level-2 inputs (needed last; done before the
            # first store chunk hits this ring)
            KC2 = LEVELS[2]["C"] // 128
            wt2_t = cpool.tile([128, KC2 * NCOL], F16, tag="wt2", name="wt2sb")
            nc.gpsimd.dma_start(out=wt2_t[:, :], in_=dram["wt2"][:, :])
            xk2 = xpool.tile([128, KC2 * LEVELS[2]["HW"]], F16, tag="x2",
                             name="xk2")
            nc.gpsimd.dma_start(out=xk2[:, :], in_=dram["x2"][:, :])
            if use_bias:
                for l in range(3):
                    b_t = cpool.tile([1, NCOL], F32, tag=f"b{l}", name=f"bt{l}")
                    nc.gpsimd.dma_start(out=b_t[:, :], in_=dram[f"b{l}"][:, :])
                    lvl[l] = b_t
            xks = [xk0, xk1, xk2]
            wts = [wt0_t, wt1_t, wt2_t]

            # resident decoded outputs; partition p element (a, t, c) is
            # output row hw = t*128+p of anchor a
            o89 = [opool.tile([128, NA, NT[l], NO], F16, tag=f"o89_{l}",
                              name=f"o89_{l}") for l in range(3)]
            o4 = [opool.tile([128, NA, NT[l], 4], F16, tag=f"o4_{l}",
                             name=f"o4_{l}") for l in range(3)]

            next_chunk = [0, 0, 0]

            def emit_chunks(l):
                L = LEVELS[l]
                done_tiles = emitted_groups[l]
                while next_chunk[l] < len(CHUNKS[l]):
                    s0, snt, qe = CHUNKS[l][next_chunk[l]]
                    if s0 + snt > done_tiles:
                        break
                    gat_c = gat_t[:, LOFF[l] + s0:LOFF[l] + s0 + snt] \
                        .transpose([0, 2, 1, 3])         # (p, a, t, c)
                    s2c = o89[l][:, :, s0:s0 + snt, 2:4]
                    # u = 1 - s ; q = s/u = exp(p) ; wh = q * anchor
                    u = spool.tile([128, NA, CHMAX, 2], F32, tag="u",
                                   name=f"u_{l}_{s0}")
                    uv = u[:, :, 0:snt]
                    nc.vector.tensor_scalar(uv, s2c, -1.0, 1.0, ALU.mult,
                                            ALU.add)
                    # whole tile (contiguous); tail beyond snt is garbage
                    uf = u.rearrange("p a t c -> p (a t c)")
                    nc.vector.reciprocal_approx_fast(uf, uf)
                    q = spool.tile([128, NA, CHMAX, 2], F32, tag="q",
                                   name=f"q_{l}_{s0}")
                    qv = q[:, :, 0:snt]
                    nc.vector.tensor_mul(qv, s2c, uv)
                    o4c = o4[l][:, :, s0:s0 + snt, :]
                    nc.vector.tensor_tensor(o4c[:, :, :, 2:4], qv,
                                            gat_c[:, :, :, 2:4], ALU.mult)
                    # xy = s*stride + grid*stride
                    nc.vector.tensor_scalar_mul(
                        o4c[:, :, :, 0:2], o89[l][:, :, s0:s0 + snt, 0:2],
                        float(L["stride"]))
                    nc.vector.tensor_add(o4c[:, :, :, 0:2], o4c[:, :, :, 0:2],
                                         gat_c[:, :, :, 0:2])

                    eng = nc.gpsimd if qe == "g" else nc.sync
                    eng.dma_start(out=dram[f"y89_{l}"][:, :, s0:s0 + snt, :],
                                  in_=o89[l][:, :, s0:s0 + snt, :])
                    eng.dma_start(out=dram[f"y4_{l}"][:, :, s0:s0 + snt, :],
                                  in_=o4[l][:, :, s0:s0 + snt, :])
                    next_chunk[l] += 1

            # ---- Phase B: one interleaved group stream ----
            emitted_groups = [0, 0, 0]
            for (l, gi) in SCHED:
                L = LEVELS[l]
                HW, stride = L["HW"], L["stride"]
                (t0, ntl, m) = GROUPS[l][gi]
                ascale = (1.0 / WSCALE) if L["fp8"] else 1.0

                ps = pspool.tile([128, GROUP, 512], F32, tag="ps",
                                 name=f"ps{l}_{t0}")
                psf = ps.rearrange("p g x -> p (g x)")
                for i in range(ntl):
                    t = t0 + i
                    if L["fp8"]:
                        KD = L["C"] // 256
                        for kd in range(KD):
                            nc.tensor.matmul(
                                psf[0:m, i * 512:i * 512 + NCOL],
                                lhsT=xks[l][:, kd, :, t * 128:t * 128 + m],
                                rhs=wts[l][:, kd, :, :],
                                start=(kd == 0),
                                stop=(kd == KD - 1 and not use_bias),
                                perf_mode=DR,
                            )
                    else:
                        KC = L["C"] // 128
                        for kc in range(KC):
                            nc.tensor.matmul(
                                psf[0:m, i * 512:i * 512 + NCOL],
                                lhsT=xks[l][:, kc * HW + t * 128:
                                            kc * HW + t * 128 + m],
                                rhs=wts[l][:, kc * NCOL:(kc + 1) * NCOL],
                                start=(kc == 0),
                                stop=(kc == KC - 1 and not use_bias),
                            )
                    if use_bias:
                        nc.tensor.matmul(
                            psf[0:m, i * 512:i * 512 + NCOL],
                            lhsT=ones_t[:, 0:m],
                            rhs=lvl[l][:, :],
                            start=False,
                            stop=True,
                        )

                # fp16 sigmoid of everything: the ONLY psum reader
                ps_a = ps[0:m, 0:ntl, 0:NCOL].rearrange(
                    "p g (a c) -> p g a c", a=NA)
                o89v = o89[l][0:m, :, t0:t0 + ntl, :].transpose([0, 2, 1, 3])
                nc.scalar.activation(o89v, ps_a, AF.Sigmoid, scale=ascale)

                emitted_groups[l] = t0 + ntl
                emit_chunks(l)

            for l in range(3):
                assert next_chunk[l] == len(CHUNKS[l]), (l, next_chunk[l])
    nc.compile()
    return nc


_PROGS = {}


def _get_prog(use_bias: bool):
    if use_bias not in _PROGS:
        _PROGS[use_bias] = _build_program(use_bias)
    return _PROGS[use_bias]


def _host_gat():
    """(128, 67, NA, 4) fp16: [...,0:2]=grid*stride, [...,2:4]=anchors."""
    gat = np.zeros((128, NTSUM, NA, 4), np.float32)
    for l, L in enumerate(LEVELS):
        HW, W, stride = L["HW"], L["W"], L["stride"]
        nt = NT[l]
        hw = np.arange(nt * 128)
        gx = (hw % W).astype(np.float32) * stride
        gy = (hw // W).astype(np.float32) * stride
        gx[HW:] = 0.0
        gy[HW:] = 0.0
        sl = gat[:, LOFF[l]:LOFF[l] + nt]
        sl[:, :, :, 0] = gx.reshape(nt, 128).T[:, :, None]
        sl[:, :, :, 1] = gy.reshape(nt, 128).T[:, :, None]
        sl[:, :, :, 2:4] = np.asarray(L["anchors"], np.float32)[None, None]
    return np.ascontiguousarray(gat.astype(np.float16))


_CONSTS = None


def _make_in_maps(xs, ws, bs, use_bias):
    global _CONSTS
    if _CONSTS is None:
        _CONSTS = _host_gat()
    wts, xps = [], []
    for l, (x, w, L) in enumerate(zip(xs, ws, LEVELS)):
        HW = L["HW"]
        if L["fp8"]:
            KD = L["C"] // 256
            # w: (267, C) -> (128, KD, 2, 267): [p,kd,j,o]=16*w[o,(kd*2+j)*128+p]
            wts.append(np.ascontiguousarray(
                (w.T * WSCALE).astype(NP_F8).reshape(KD, 2, 128, NCOL)
                .transpose(2, 0, 1, 3)))
            # x: (B, C, HW) -> (B, 128, KD, 2, HW): [p,kd,j,hw]=x[(kd*2+j)*128+p]
            xps.append(np.ascontiguousarray(
                x.reshape(NCORES, KD, 2, 128, HW).astype(NP_F8)
                .transpose(0, 3, 1, 2, 4)))
        else:
            KC = L["C"] // 128
            wts.append(np.ascontiguousarray(
                w.T.astype(np.float16).reshape(KC, 128, NCOL)
                .transpose(1, 0, 2).reshape(128, KC * NCOL)))
            xps.append(np.ascontiguousarray(
                x.reshape(NCORES, KC, 128, HW).astype(np.float16)
                .transpose(0, 2, 1, 3).reshape(NCORES, 128, KC * HW)))
    in_maps = []
    for core in range(NCORES):
        im = {"gat": _CONSTS}
        for l in range(len(LEVELS)):
            im[f"x{l}"] = xps[l][core]
            im[f"wt{l}"] = wts[l]
            if use_bias:
                scale = WSCALE if LEVELS[l]["fp8"] else 1.0
                im[f"b{l}"] = np.ascontiguousarray(
                    (bs[l] * scale).reshape(1, NCOL).astype(np.float32))
        in_maps.append(im)
    return in_maps


def _assemble(results):
    """y89 (128,NA,nt,89) + y4 (128,NA,nt,4) fp16 -> (8, 25200, 89) fp32."""
    out = np.empty((NCORES, 25200, NO), np.float32)
    for core in range(NCORES):
        parts = []
        for l, L in enumerate(LEVELS):
            HW = L["HW"]
            nt = NT[l]
            y89 = results[core][f"y89_{l}"].astype(np.float32)
            y4 = results[core][f"y4_{l}"].astype(np.float32)
            y = y89.transpose(1, 2, 0, 3).reshape(NA, nt * 128, NO)[:, :HW, :]
            y4t = y4.transpose(1, 2, 0, 3).reshape(NA, nt * 128, 4)[:, :HW, :]
            y[:, :, 0:4] = y4t
            parts.append(y.reshape(NA * HW, NO))
        out[core] = np.concatenate(parts, axis=0)
    return out


def _run(x0, x1, x2, w0, b0, w1, b1, w2, b2, **spmd_kwargs):
    xs = [np.asarray(x, dtype=np.float32) for x in (x0, x1, x2)]
    ws = [np.asarray(w, dtype=np.float32) for w in (w0, w1, w2)]
    bs = [np.asarray(b, dtype=np.float32) for b in (b0, b1, b2)]
    use_bias = any(np.any(b != 0) for b in bs)
    in_maps = _make_in_maps(xs, ws, bs, use_bias)
    res = run_bass_kernel_spmd(_get_prog(use_bias), in_maps,
                               core_ids=list(range(NCORES)), **spmd_kwargs)
    return _assemble(res.results), res


def kernel(x0, x1, x2, w0, b0, w1, b1, w2, b2):
    out, _ = _run(x0, x1, x2, w0, b0, w1, b1, w2, b2)
    return out


def kernel_traced(x0, x1, x2, w0, b0, w1, b1, w2, b2):
    """Like kernel() but with NTFF tracing; returns (out, BassKernelResults)."""
    return _run(x0, x1, x2, w0, b0, w1, b1, w2, b2, trace=True)


# revision 15
# speedup vs baseline: 1.0913x; 1.0913x over previous
"""YOLO-detect head (1x1 conv + box decode) on 8 Trainium2 NeuronCores.

Data-parallel over batch: core b processes batch element b.

Per core, per level l (C channels, HW = ny*nx positions):
  p[hw, o] = sum_c x[c, hw] * w[o, c]      (o = a*89 + ch, a anchor, ch channel)
computed on the tensor engine as out = lhsT.T @ rhs with
  lhsT = x chunk  [K channels, M<=128 hw]   (stationary)
  rhs  = w.T chunk [K channels, N=267]      (moving)
so the PSUM result is already [hw, 267] - no on-chip transpose.
Levels 0-1 run in fp8(e4m3) DoubleRow mode: K=256 contracted per instruction
(w0/w1 host-prescaled by 16 to clear e4m3's subnormal range; compensated by
the activation input scale 1/16). Level 2 stays fp16 (its biggest anchors
amplify exp() error, so it keeps the precision).

Decode: the ACT engine uses the sigmoid table DIRECTLY (one fp16 sigmoid of
all 89 cols per psum group; the ONLY psum reader, so psum groups free as
early as possible). The exp table is never needed -> one table load total.
  wh:  u = 1-s (DVE), u <- 1/u (fast approx reciprocal), q = s*u = exp(p),
       wh = q*anchor
  xy:  xy = s*stride + grid*stride
These fixups read the RESIDENT sigmoid tile (not psum) and therefore run
per ~8-12-tile store chunk, off the psum critical path.

Schedule: levels are interleaved into one global group stream ordered by
input-arrival time (L0 first, L1 woven in from slot 6, L2 near the end) so
the scalar engine's sigmoid stream - the throughput floor of this kernel -
never starves while matmuls for later levels fill PE idle slots.

DMA: x0 pieces load on nc.sync's HWDGE ring; everything else loads on
nc.vector's ring in parallel (descriptor generation is ~0.65us each and
serializes per-engine). Early big store chunks go via nc.gpsimd (SWDGE);
final small chunks via nc.sync so the drain tail is short.
"""

import numpy as np
import ml_dtypes

import concourse.bacc as bacc
import concourse.mybir as mybir
import concourse.tile as tile
from concourse.bass_utils import run_bass_kernel_spmd

F32 = mybir.dt.float32
F16 = mybir.dt.float16
F8 = mybir.dt.float8e4
AF = mybir.ActivationFunctionType
ALU = mybir.AluOpType
DR = mybir.MatmulPerfMode.DoubleRow
NP_F8 = ml_dtypes.float8_e4m3fn

NCORES = 8
NA = 3          # anchors per level
NO = 89         # channels per anchor (80 classes + 5 + 4)
NCOL = NA * NO  # 267
GROUP = 4       # full 128-row hw tiles per PSUM group (4 banks; 2 bufs = all 8)
WSCALE = 16.0   # host pre-scale on fp8 weights (subnormal avoidance)

LEVELS = [
    dict(C=256,  W=80, HW=6400, stride=8.0, fp8=True,
         anchors=((10.0, 13.0), (16.0, 30.0), (33.0, 23.0))),
    dict(C=512,  W=40, HW=1600, stride=16.0, fp8=True,
         anchors=((30.0, 61.0), (62.0, 45.0), (59.0, 119.0))),
    dict(C=1024, W=20, HW=400,  stride=32.0, fp8=False,
         anchors=((116.0, 90.0), (156.0, 198.0), (373.0, 326.0))),
]
NT = [(L["HW"] + 127) // 128 for L in LEVELS]   # 50, 13, 4
NTSUM = sum(NT)                                  # 67
LOFF = [sum(NT[:l]) for l in range(3)]

# (t0, ntiles, rows) per level
GROUPS = []
for _l, _L in enumerate(LEVELS):
    full, rem = divmod(_L["HW"], 128)
    g = []
    t0 = 0
    while t0 < full:
        n = min(GROUP, full - t0)
        g.append((t0, n, 128))
        t0 += n
    if rem:
        g.append((full, 1, rem))
    GROUPS.append(g)

# global schedule: (level, group_idx); L1 woven in from slot 8 (after its
# input lands), L2 near the end, its tiny partial group dead last
SCHED = ([(0, i) for i in range(8)]
         + [(1, 0), (0, 8), (1, 1), (0, 9), (1, 2), (0, 10), (1, 3),
            (0, 11), (2, 0), (0, 12), (2, 1)])
assert sorted(SCHED) == sorted(
    (l, i) for l in range(3) for i in range(len(GROUPS[l])))

# store chunks (t0, ntiles, engine): big ones on gpsimd SWDGE (sustains
# ~270 B/ns vs ~130 for HWDGE stores), late small ones on sync so the two
# queues flush the tail in parallel
CHUNKS = [
    [(0, 12, "g"), (12, 12, "g"), (24, 12, "g"), (36, 8, "g"), (44, 6, "s")],
    [(0, 8, "g"), (8, 5, "s")],
    [(0, 4, "g")],
]
CHMAX = 12


def _build_program(use_bias: bool):
    # Bacc (not raw Bass): its compile() runs move_matmul_waits_to_ldweights +
    # generate_event_semaphores, without which walrus rejects instructions
    # that carry more than one semaphore wait.
    nc = bacc.Bacc("TRN2", target_bir_lowering=False, debug=False)

    dram = {}
    for l, L in enumerate(LEVELS):
        KD = L["C"] // 256
        if L["fp8"]:
            dram[f"x{l}"] = nc.dram_tensor(f"x{l}", (128, KD, 2, L["HW"]), F8,
                                           kind="ExternalInput").ap()
            dram[f"wt{l}"] = nc.dram_tensor(f"wt{l}", (128, KD, 2, NCOL), F8,
                                            kind="ExternalInput").ap()
        else:
            KC = L["C"] // 128
            dram[f"x{l}"] = nc.dram_tensor(f"x{l}", (128, KC * L["HW"]), F16,
                                           kind="ExternalInput").ap()
            dram[f"wt{l}"] = nc.dram_tensor(f"wt{l}", (128, KC * NCOL), F16,
                                            kind="ExternalInput").ap()
        nt = NT[l]
        dram[f"y89_{l}"] = nc.dram_tensor(f"y89_{l}", (128, NA, nt, NO), F16,
                                          kind="ExternalOutput").ap()
        dram[f"y4_{l}"] = nc.dram_tensor(f"y4_{l}", (128, NA, nt, 4), F16,
                                         kind="ExternalOutput").ap()
        if use_bias:
            dram[f"b{l}"] = nc.dram_tensor(f"b{l}", (1, NCOL), F32,
                                           kind="ExternalInput").ap()
    # gat[p, t, a, 0:2] = grid*stride for hw row t*128+p (replicated over a)
    # gat[p, t, a, 2:4] = anchor wh (replicated over t)
    dram["gat"] = nc.dram_tensor("gat", (128, NTSUM, NA, 4), F16,
                                 kind="ExternalInput").ap()

    with tile.TileContext(nc) as tc:
        with tc.tile_pool(name="consts", bufs=1) as cpool, \
             tc.tile_pool(name="xbuf", bufs=1) as xpool, \
             tc.tile_pool(name="obuf", bufs=1) as opool, \
             tc.tile_pool(name="scr", bufs=2) as spool, \
             tc.tile_pool(name="ps", bufs=2, space="PSUM") as pspool:

            ones_t = None
            if use_bias:
                ones_t = cpool.tile([1, 128], F16, tag="ones", name="ones")
                nc.vector.memset(ones_t[:, :], 1.0)

            # ---- Phase A: loads serial on sync in need-order (anything
            # running in parallel early steals bandwidth from x0 and starves
            # the level-0 stream); tiny gat on the otherwise-idle SWDGE ring
            lvl = {}
            wt0_t = cpool.tile([128, 1, 2, NCOL], F8, tag="wt0", name="wt0sb")
            nc.sync.dma_start(out=wt0_t[:, :, :, :], in_=dram["wt0"][:, :, :, :])
            xk0 = xpool.tile([128, 1, 2, LEVELS[0]["HW"]], F8, tag="x0",
                             name="xk0")
            for (c0, c1) in ((0, 1024), (1024, 3072), (3072, LEVELS[0]["HW"])):
                nc.sync.dma_start(out=xk0[:, :, :, c0:c1],
                                  in_=dram["x0"][:, :, :, c0:c1])
            gat_t = cpool.tile([128, NTSUM, NA, 4], F16, tag="gat",
                               name="gatsb")
            nc.gpsimd.dma_start(out=gat_t[:, :, :, :], in_=dram["gat"][:, :, :, :])
            wt1_t = cpool.tile([128, 2, 2, NCOL], F8, tag="wt1", name="wt1sb")
            nc.sync.dma_start(out=wt1_t[:, :, :, :], in_=dram["wt1"][:, :, :, :])
            xk1 = xpool.tile([128, 2, 2, LEVELS[1]["HW"]], F8, tag="x1",
                             name="xk1")
            nc.sync.dma_start(out=xk1[:, :, :, :], in_=dram["x1"][:, :, :, :])
            KC2 = LEVELS[2]["C"] // 128
            wt2_t = cpool.tile([128, KC2 * NCOL], F16, tag="wt2", name="wt2sb")
            nc.sync.dma_start(out=wt2_t[:, :], in_=dram["wt2"][:, :])
            xk2 = xpool.tile([128, KC2 * LEVELS[2]["HW"]], F16, tag="x2",
                             name="xk2")
            nc.sync.dma_start(out=xk2[:, :], in_=dram["x2"][:, :])
            if use_bias:
                for l in range(3):
                    b_t = cpool.tile([1, NCOL], F32, tag=f"b{l}", name=f"bt{l}")
                    nc.gpsimd.dma_start(out=b_t[:, :], in_=dram[f"b{l}"][:, :])
                    lvl[l] = b_t
            xks = [xk0, xk1, xk2]
            wts = [wt0_t, wt1_t, wt2_t]

            # resident decoded outputs; partition p element (a, t, c) is
            # output row hw = t*128+p of anchor a
            o89 = [opool.tile([128, NA, NT[l], NO], F16, tag=f"o89_{l}",
                              name=f"o89_{l}") for l in range(3)]
            o4 = [opool.tile([128, NA, NT[l], 4], F16, tag=f"o4_{l}",
                             name=f"o4_{l}") for l in range(3)]

            next_chunk = [0, 0, 0]

            def emit_chunks(l):
                L = LEVELS[l]
                done_tiles = emitted_groups[l]
                while next_chunk[l] < len(CHUNKS[l]):
                    s0, snt, qe = CHUNKS[l][next_chunk[l]]
                    if s0 + snt > done_tiles:
                        break
                    gat_c = gat_t[:, LOFF[l] + s0:LOFF[l] + s0 + snt] \
                        .transpose([0, 2, 1, 3])         # (p, a, t, c)
                    s2c = o89[l][:, :, s0:s0 + snt, 2:4]
                    # u = 1 - s ; q = s/u = exp(p) ; wh = q * anchor
                    u = spool.tile([128, NA, CHMAX, 2], F32, tag="u",
                                   name=f"u_{l}_{s0}")
                    uv = u[:, :, 0:snt]
                    nc.vector.tensor_scalar(uv, s2c, -1.0, 1.0, ALU.mult,
                                            ALU.add)
                    # whole tile (contiguous); tail beyond snt is garbage
                    uf = u.rearrange("p a t c -> p (a t c)")
                    nc.vector.reciprocal_approx_fast(uf, uf)
                    q = spool.tile([128, NA, CHMAX, 2], F32, tag="q",
                                   name=f"q_{l}_{s0}")
                    qv = q[:, :, 0:snt]
                    nc.vector.tensor_mul(qv, s2c, uv)
                    o4c = o4[l][:, :, s0:s0 + snt, :]
                    nc.vector.tensor_tensor(o4c[:, :, :, 2:4], qv,
                                            gat_c[:, :, :, 2:4], ALU.mult)
                    # xy = s*stride + grid*stride
                    nc.vector.tensor_scalar_mul(
                        o4c[:, :, :, 0:2], o89[l][:, :, s0:s0 + snt, 0:2],
                        float(L["stride"]))
                    nc.vector.tensor_add(o4c[:, :, :, 0:2], o4c[:, :, :, 0:2],
                                         gat_c[:, :, :, 0:2])

                    eng = nc.gpsimd if qe == "g" else nc.sync
                    eng.dma_start(out=dram[f"y89_{l}"][:, :, s0:s0 + snt, :],
                                  in_=o89[l][:, :, s0:s0 + snt, :])
                    eng.dma_start(out=dram[f"y4_{l}"][:, :, s0:s0 + snt, :],
                                  in_=o4[l][:, :, s0:s0 + snt, :])
                    next_chunk[l] += 1

            # ---- Phase B: one interleaved group stream ----
            emitted_groups = [0, 0, 0]
            for (l, gi) in SCHED:
                L = LEVELS[l]
                HW, stride = L["HW"], L["stride"]
                (t0, ntl, m) = GROUPS[l][gi]
                ascale = (1.0 / WSCALE) if L["fp8"] else 1.0

                ps = pspool.tile([128, GROUP, 512], F32, tag="ps",
                                 name=f"ps{l}_{t0}")
                psf = ps.rearrange("p g x -> p (g x)")
                for i in range(ntl):
                    t = t0 + i
                    if L["fp8"]:
                        KD = L["C"] // 256
                        for kd in range(KD):
                            nc.tensor.matmul(
                                psf[0:m, i * 512:i * 512 + NCOL],
                                lhsT=xks[l][:, kd, :, t * 128:t * 128 + m],
                                rhs=wts[l][:, kd, :, :],
                                start=(kd == 0),
                                stop=(kd == KD - 1 and not use_bias),
                                perf_mode=DR,
                            )
                    else:
                        KC = L["C"] // 128
                        for kc in range(KC):
                            nc.tensor.matmul(
                                psf[0:m, i * 512:i * 512 + NCOL],
                                lhsT=xks[l][:, kc * HW + t * 128:
                                            kc * HW + t * 128 + m],
                                rhs=wts[l][:, kc * NCOL:(kc + 1) * NCOL],
                                start=(kc == 0),
                                stop=(kc == KC - 1 and not use_bias),
                            )
                    if use_bias:
                        nc.tensor.matmul(
                            psf[0:m, i * 512:i * 512 + NCOL],
                            lhsT=ones_t[:, 0:m],
                            rhs=lvl[l][:, :],
                            start=False,
                            stop=True,
                        )

                # fp16 sigmoid of everything: the ONLY psum reader
                ps_a = ps[0:m, 0:ntl, 0:NCOL].rearrange(
                    "p g (a c) -> p g a c", a=NA)
                o89v = o89[l][0:m, :, t0:t0 + ntl, :].transpose([0, 2, 1, 3])
                nc.scalar.activation(o89v, ps_a, AF.Sigmoid, scale=ascale)

                emitted_groups[l] = t0 + ntl
                emit_chunks(l)

            for l in range(3):
                assert next_chunk[l] == len(CHUNKS[l]), (l, next_chunk[l])
    nc.compile()
    return nc


_PROGS = {}


def _get_prog(use_bias: bool):
    if use_bias not in _PROGS:
        _PROGS[use_bias] = _build_program(use_bias)
    return _PROGS[use_bias]


def _host_gat():
    """(128, 67, NA, 4) fp16: [...,0:2]=grid*stride, [...,2:4]=anchors."""
    gat = np.zeros((128, NTSUM, NA, 4), np.float32)
    for l, L in enumerate(LEVELS):
        HW, W, stride = L["HW"], L["W"], L["stride"]
        nt = NT[l]
        hw = np.arange(nt * 128)
        gx = (hw % W).astype(np.float32) * stride
        gy = (hw // W).astype(np.float32) * stride
        gx[HW:] = 0.0
        gy[HW:] = 0.0
        sl = gat[:, LOFF[l]:LOFF[l] + nt]
        sl[:, :, :, 0] = gx.reshape(nt, 128).T[:, :, None]
        sl[:, :, :, 1] = gy.reshape(nt, 128).T[:, :, None]
        sl[:, :, :, 2:4] = np.asarray(L["anchors"], np.float32)[None, None]
    return np.ascontiguousarray(gat.astype(np.float16))


_CONSTS = None


def _make_in_maps(xs, ws, bs, use_bias):
    global _CONSTS
    if _CONSTS is None:
        _CONSTS = _host_gat()
    wts, xps = [], []
    for l, (x, w, L) in enumerate(zip(xs, ws, LEVELS)):
        HW = L["HW"]
        if L["fp8"]:
            KD = L["C"] // 256
            # w: (267, C) -> (128, KD, 2, 267): [p,kd,j,o]=16*w[o,(kd*2+j)*128+p]
            wts.append(np.ascontiguousarray(
                (w.T * WSCALE).astype(NP_F8).reshape(KD, 2, 128, NCOL)
                .transpose(2, 0, 1, 3)))
            # x: (B, C, HW) -> (B, 128, KD, 2, HW): [p,kd,j,hw]=x[(kd*2+j)*128+p]
            xps.append(np.ascontiguousarray(
                x.reshape(NCORES, KD, 2, 128, HW).astype(NP_F8)
                .transpose(0, 3, 1, 2, 4)))
        else:
            KC = L["C"] // 128
            wts.append(np.ascontiguousarray(
                w.T.astype(np.float16).reshape(KC, 128, NCOL)
                .transpose(1, 0, 2).reshape(128, KC * NCOL)))
            xps.append(np.ascontiguousarray(
                x.reshape(NCORES, KC, 128, HW).astype(np.float16)
                .transpose(0, 2, 1, 3).reshape(NCORES, 128, KC * HW)))
    in_maps = []
    for core in range(NCORES):
        im = {"gat": _CONSTS}
        for l in range(len(LEVELS)):
            im[f"x{l}"] = xps[l][core]
            im[f"wt{l}"] = wts[l]
            if use_bias:
                scale = WSCALE if LEVELS[l]["fp8"] else 1.0
                im[f"b{l}"] = np.ascontiguousarray(
                    (bs[l] * scale).reshape(1, NCOL).astype(np.float32))
        in_maps.append(im)
    return in_maps


def _assemble(results):
    """y89 (128,NA,nt,89) + y4 (128,NA,nt,4) fp16 -> (8, 25200, 89) fp32."""
    out = np.empty((NCORES, 25200, NO), np.float32)
    for core in range(NCORES):
        parts = []
        for l, L in enumerate(LEVELS):
            HW = L["HW"]
            nt = NT[l]
            y89 = results[core][f"y89_{l}"].astype(np.float32)
            y4 = results[core][f"y4_{l}"].astype(np.float32)
            y = y89.transpose(1, 2, 0, 3).reshape(NA, nt * 128, NO)[:, :HW, :]
            y4t = y4.transpose(1, 2, 0, 3).reshape(NA, nt * 128, 4)[:, :HW, :]
            y[:, :, 0:4] = y4t
            parts.append(y.reshape(NA * HW, NO))
        out[core] = np.concatenate(parts, axis=0)
    return out


def _run(x0, x1, x2, w0, b0, w1, b1, w2, b2, **spmd_kwargs):
    xs = [np.asarray(x, dtype=np.float32) for x in (x0, x1, x2)]
    ws = [np.asarray(w, dtype=np.float32) for w in (w0, w1, w2)]
    bs = [np.asarray(b, dtype=np.float32) for b in (b0, b1, b2)]
    use_bias = any(np.any(b != 0) for b in bs)
    in_maps = _make_in_maps(xs, ws, bs, use_bias)
    res = run_bass_kernel_spmd(_get_prog(use_bias), in_maps,
                               core_ids=list(range(NCORES)), **spmd_kwargs)
    return _assemble(res.results), res


def kernel(x0, x1, x2, w0, b0, w1, b1, w2, b2):
    out, _ = _run(x0, x1, x2, w0, b0, w1, b1, w2, b2)
    return out


def kernel_traced(x0, x1, x2, w0, b0, w1, b1, w2, b2):
    """Like kernel() but with NTFF tracing; returns (out, BassKernelResults)."""
    return _run(x0, x1, x2, w0, b0, w1, b1, w2, b2, trace=True)


# revision 17
# speedup vs baseline: 1.1541x; 1.0575x over previous
"""YOLO-detect head (1x1 conv + box decode) on 8 Trainium2 NeuronCores.

Data-parallel over batch: core b processes batch element b.

Per core, per level l (C channels, HW = ny*nx positions):
  p[hw, o] = sum_c x[c, hw] * w[o, c]      (o = a*89 + ch, a anchor, ch channel)
computed on the tensor engine as out = lhsT.T @ rhs with
  lhsT = x chunk  [K channels, M<=128 hw]   (stationary)
  rhs  = w.T chunk [K channels, N=267]      (moving)
so the PSUM result is already [hw, 267] - no on-chip transpose.
Levels 0-1 run in fp8(e4m3) DoubleRow mode: K=256 contracted per instruction
(w0/w1 host-prescaled by 16 to clear e4m3's subnormal range; compensated by
the activation input scale 1/16). Level 2 stays fp16 (its biggest anchors
amplify exp() error, so it keeps the precision).

Decode uses the sigmoid ACT table directly (exp never needed -> one table
load total). Per psum group, on the scalar engine:
  ACT_a: fp32 sigmoid of cols 0:4 -> resident s4 tile  (xy/wh precision)
  ACT_b: fp8 sigmoid of cols 4:89 -> resident y85 tile (the store payload;
         fp8 of a value in (0,1) is ~0.03 abs, deep inside tolerance)
The xy/wh fixups read the RESIDENT s4 (not psum), so they run per
~12-14-tile store chunk on the DVE, off the psum critical path:
  u = 1-s; u <- 1/u (fast approx reciprocal); u <- s*u (= exp(p));
  wh = u*anchor;  xy = s*stride + grid*stride.

Schedule: levels interleave into one global group stream ordered by input
arrival (L0 first, L1 woven from slot 8, L2 at the end) so the scalar
engine's sigmoid stream - the throughput floor - never starves while later
levels' matmuls fill PE slack.

DMA: loads are serial on nc.sync's HWDGE ring in need-order (parallel rings
just steal bandwidth from x0 and starve the stream start); each level's
weights+activations are merged into ONE dram tensor so descriptor-gen and
semaphore count stay low. Stores: fp8 y85 + small fp16 y4 per chunk, big
chunks via nc.gpsimd (SWDGE sustains ~270 B/ns vs ~130 for HWDGE stores),
final chunks split across both rings so the tail flushes in parallel.
"""

import numpy as np
import ml_dtypes

import concourse.bacc as bacc
import concourse.mybir as mybir
import concourse.tile as tile
from concourse.bass_utils import run_bass_kernel_spmd

F32 = mybir.dt.float32
F16 = mybir.dt.float16
F8 = mybir.dt.float8e4
AF = mybir.ActivationFunctionType
ALU = mybir.AluOpType
DR = mybir.MatmulPerfMode.DoubleRow
NP_F8 = ml_dtypes.float8_e4m3fn

NCORES = 8
NA = 3          # anchors per level
NO = 89         # channels per anchor (80 classes + 5 + 4)
NCOL = NA * NO  # 267
GROUP = 4       # full 128-row hw tiles per PSUM group (4 banks; 2 bufs = all 8)
WSCALE = 16.0   # host pre-scale on fp8 weights (subnormal avoidance)
WPAD = 272      # fp8 weight block padded so x starts at an even offset (DR ldweights alignment)

LEVELS = [
    dict(C=256,  W=80, HW=6400, stride=8.0, fp8=True,
         anchors=((10.0, 13.0), (16.0, 30.0), (33.0, 23.0))),
    dict(C=512,  W=40, HW=1600, stride=16.0, fp8=True,
         anchors=((30.0, 61.0), (62.0, 45.0), (59.0, 119.0))),
    dict(C=1024, W=20, HW=400,  stride=32.0, fp8=False,
         anchors=((116.0, 90.0), (156.0, 198.0), (373.0, 326.0))),
]
NT = [(L["HW"] + 127) // 128 for L in LEVELS]   # 50, 13, 4
NTSUM = sum(NT)                                  # 67
LOFF = [sum(NT[:l]) for l in range(3)]

# (t0, ntiles, rows) per level
GROUPS = []
for _l, _L in enumerate(LEVELS):
    full, rem = divmod(_L["HW"], 128)
    g = []
    t0 = 0
    while t0 < full:
        n = min(GROUP, full - t0)
        g.append((t0, n, 128))
        t0 += n
    if rem:
        g.append((full, 1, rem))
    GROUPS.append(g)

# global schedule: (level, group_idx); L1 woven in from slot 8 (after its
# input lands), L2 near the end, its tiny partial group dead last
SCHED = ([(0, i) for i in range(8)]
         + [(1, 0), (0, 8), (1, 1), (0, 9), (1, 2), (0, 10), (1, 3),
            (0, 11), (2, 0), (0, 12), (2, 1)])
assert sorted(SCHED) == sorted(
    (l, i) for l in range(3) for i in range(len(GROUPS[l])))

# store chunks (t0, ntiles, engine): big ones on gpsimd SWDGE, the late ones
# split across rings so the tail flushes in parallel
CHUNKS = [
    [(0, 12, "g"), (12, 12, "g"), (24, 12, "g"), (36, 14, "s")],
    [(0, 8, "g"), (8, 5, "s")],
    [(0, 4, "g")],
]
CHMAX = 14


def _build_program(use_bias: bool):
    # Bacc (not raw Bass): its compile() runs move_matmul_waits_to_ldweights +
    # generate_event_semaphores, without which walrus rejects instructions
    # that carry more than one semaphore wait.
    nc = bacc.Bacc("TRN2", target_bir_lowering=False, debug=False)

    dram = {}
    # merged per-level [weights | activations] input tensors
    dram["wx0"] = nc.dram_tensor("wx0", (128, 1, 2, WPAD + LEVELS[0]["HW"]),
                                 F8, kind="ExternalInput").ap()
    dram["wx1"] = nc.dram_tensor("wx1", (128, 2, 2, WPAD + LEVELS[1]["HW"]),
                                 F8, kind="ExternalInput").ap()
    KC2 = LEVELS[2]["C"] // 128
    dram["wx2"] = nc.dram_tensor("wx2", (128, KC2 * (NCOL + LEVELS[2]["HW"])),
                                 F16, kind="ExternalInput").ap()
    for l in range(3):
        nt = NT[l]
        dram[f"y85_{l}"] = nc.dram_tensor(f"y85_{l}", (128, NA, nt, 85), F8,
                                          kind="ExternalOutput").ap()
        dram[f"y4_{l}"] = nc.dram_tensor(f"y4_{l}", (128, NA, nt, 4), F16,
                                         kind="ExternalOutput").ap()
        if use_bias:
            dram[f"b{l}"] = nc.dram_tensor(f"b{l}", (1, NCOL), F32,
                                           kind="ExternalInput").ap()
    # gat[p, t, a, 0:2] = grid*stride for hw row t*128+p (replicated over a)
    # gat[p, t, a, 2:4] = anchor wh (replicated over t)
    dram["gat"] = nc.dram_tensor("gat", (128, NTSUM, NA, 4), F16,
                                 kind="ExternalInput").ap()

    with tile.TileContext(nc) as tc:
        with tc.tile_pool(name="consts", bufs=1) as cpool, \
             tc.tile_pool(name="xbuf", bufs=1) as xpool, \
             tc.tile_pool(name="obuf", bufs=1) as opool, \
             tc.tile_pool(name="scr", bufs=2) as spool, \
             tc.tile_pool(name="ps", bufs=2, space="PSUM") as pspool:

            ones_t = None
            if use_bias:
                ones_t = cpool.tile([1, 128], F16, tag="ones", name="ones")
                nc.vector.memset(ones_t[:, :], 1.0)

            # ---- Phase A: loads serial on sync in need-order (anything
            # running in parallel early steals bandwidth from x0 and starves
            # the level-0 stream); tiny gat on the otherwise-idle SWDGE ring
            lvl = {}
            HW0 = LEVELS[0]["HW"]
            wx0 = xpool.tile([128, 1, 2, WPAD + HW0], F8, tag="wx0",
                             name="wx0sb")
            for (c0, c1) in ((0, WPAD + 1024), (WPAD + 1024, WPAD + 3072),
                             (WPAD + 3072, WPAD + HW0)):
                nc.sync.dma_start(out=wx0[:, :, :, c0:c1],
                                  in_=dram["wx0"][:, :, :, c0:c1])
            gat_t = cpool.tile([128, NTSUM, NA, 4], F16, tag="gat",
                               name="gatsb")
            nc.gpsimd.dma_start(out=gat_t[:, :, :, :], in_=dram["gat"][:, :, :, :])
            HW1 = LEVELS[1]["HW"]
            wx1 = xpool.tile([128, 2, 2, WPAD + HW1], F8, tag="wx1",
                             name="wx1sb")
            nc.sync.dma_start(out=wx1[:, :, :, :], in_=dram["wx1"][:, :, :, :])
            HW2 = LEVELS[2]["HW"]
            wx2 = xpool.tile([128, KC2 * (NCOL + HW2)], F16, tag="wx2",
                             name="wx2sb")
            nc.sync.dma_start(out=wx2[:, :], in_=dram["wx2"][:, :])
            if use_bias:
                for l in range(3):
                    b_t = cpool.tile([1, NCOL], F32, tag=f"b{l}", name=f"bt{l}")
                    nc.gpsimd.dma_start(out=b_t[:, :], in_=dram[f"b{l}"][:, :])
                    lvl[l] = b_t

            # weight / activation views inside the merged tiles
            wts = [wx0[:, :, :, 0:NCOL], wx1[:, :, :, 0:NCOL], None]
            xs8 = [wx0[:, :, :, WPAD:], wx1[:, :, :, WPAD:], None]
            wx2v = wx2.rearrange("p (k c) -> p k c", k=KC2)
            wt2v = wx2v[:, :, 0:NCOL]
            x2v = wx2v[:, :, NCOL:]

            # resident tiles; partition p element (a, t, c) is output row
            # hw = t*128+p of anchor a
            y85 = [opool.tile([128, NA, NT[l], 85], F8, tag=f"y85_{l}",
                              name=f"y85_{l}") for l in range(3)]
            o4 = [opool.tile([128, NA, NT[l], 4], F16, tag=f"o4_{l}",
                             name=f"o4_{l}") for l in range(3)]
            s4 = [opool.tile([128, NA, NT[l], 4], F32, tag=f"s4_{l}",
                             name=f"s4_{l}") for l in range(3)]

            next_chunk = [0, 0, 0]
            emitted_groups = [0, 0, 0]

            def emit_chunks(l):
                L = LEVELS[l]
                while next_chunk[l] < len(CHUNKS[l]):
                    s0, snt, qe = CHUNKS[l][next_chunk[l]]
                    if s0 + snt > emitted_groups[l]:
                        break
                    gat_c = gat_t[:, LOFF[l] + s0:LOFF[l] + s0 + snt] \
                        .transpose([0, 2, 1, 3])           # (p, a, t, c)
                    s2c = s4[l][:, :, s0:s0 + snt, 2:4]    # fp32 sigmoid
                    # u = 1-s ; u <- 1/u ; u <- s*u (= exp(p)) ; wh = u*anchor
                    u = spool.tile([128, NA, CHMAX, 2], F32, tag="u",
                                   name=f"u_{l}_{s0}")
                    uv = u[:, :, 0:snt]
                    nc.vector.tensor_scalar(uv, s2c, -1.0, 1.0, ALU.mult,
                                            ALU.add)
                    # whole tile (contiguous); tail beyond snt is garbage
                    uf = u.rearrange("p a t c -> p (a t c)")
                    nc.vector.reciprocal_approx_fast(uf, uf)
                    nc.vector.tensor_mul(uv, s2c, uv)
                    o4c = o4[l][:, :, s0:s0 + snt, :]
                    nc.vector.tensor_tensor(o4c[:, :, :, 2:4], uv,
                                            gat_c[:, :, :, 2:4], ALU.mult)
                    # xy = s*stride + grid*stride
                    nc.vector.tensor_scalar_mul(
                        o4c[:, :, :, 0:2], s4[l][:, :, s0:s0 + snt, 0:2],
                        float(L["stride"]))
                    nc.vector.tensor_add(o4c[:, :, :, 0:2], o4c[:, :, :, 0:2],
                                         gat_c[:, :, :, 0:2])

                    eng = nc.gpsimd if qe == "g" else nc.sync
                    eng.dma_start(out=dram[f"y85_{l}"][:, :, s0:s0 + snt, :],
                                  in_=y85[l][:, :, s0:s0 + snt, :])
                    eng.dma_start(out=dram[f"y4_{l}"][:, :, s0:s0 + snt, :],
                                  in_=o4[l][:, :, s0:s0 + snt, :])
                    next_chunk[l] += 1

            # ---- Phase B: one interleaved group stream ----
            for (l, gi) in SCHED:
                L = LEVELS[l]
                (t0, ntl, m) = GROUPS[l][gi]
                ascale = (1.0 / WSCALE) if L["fp8"] else 1.0

                ps = pspool.tile([128, GROUP, 512], F32, tag="ps",
                                 name=f"ps{l}_{t0}")
                psf = ps.rearrange("p g x -> p (g x)")
                for i in range(ntl):
                    t = t0 + i
                    if L["fp8"]:
                        KD = L["C"] // 256
                        for kd in range(KD):
                            nc.tensor.matmul(
                                psf[0:m, i * 512:i * 512 + NCOL],
                                lhsT=xs8[l][:, kd, :, t * 128:t * 128 + m],
                                rhs=wts[l][:, kd, :, :],
                                start=(kd == 0),
                                stop=(kd == KD - 1 and not use_bias),
                                perf_mode=DR,
                            )
                    else:
                        for kc in range(KC2):
                            nc.tensor.matmul(
                                psf[0:m, i * 512:i * 512 + NCOL],
                                lhsT=x2v[:, kc, t * 128:t * 128 + m],
                                rhs=wt2v[:, kc, :],
                                start=(kc == 0),
                                stop=(kc == KC2 - 1 and not use_bias),
                            )
                    if use_bias:
                        nc.tensor.matmul(
                            psf[0:m, i * 512:i * 512 + NCOL],
                            lhsT=ones_t[:, 0:m],
                            rhs=lvl[l][:, :],
                            start=False,
                            stop=True,
                        )

                ps_a = ps[0:m, 0:ntl, 0:NCOL].rearrange(
                    "p g (a c) -> p g a c", a=NA)
                # small fp32 sigmoid first (xy/wh source), then the big fp8
                # sigmoid whose completion releases the psum group
                s4v = s4[l][0:m, :, t0:t0 + ntl, :].transpose([0, 2, 1, 3])
                nc.scalar.activation(s4v, ps_a[:, :, :, 0:4], AF.Sigmoid,
                                     scale=ascale)
                y85v = y85[l][0:m, :, t0:t0 + ntl, :].transpose([0, 2, 1, 3])
                nc.scalar.activation(y85v, ps_a[:, :, :, 4:NO], AF.Sigmoid,
                                     scale=ascale)

                emitted_groups[l] = t0 + ntl
                emit_chunks(l)

            for l in range(3):
                assert next_chunk[l] == len(CHUNKS[l]), (l, next_chunk[l])
    nc.compile()
    return nc


_PROGS = {}


def _get_prog(use_bias: bool):
    if use_bias not in _PROGS:
        _PROGS[use_bias] = _build_program(use_bias)
    return _PROGS[use_bias]


def _host_gat():
    """(128, 67, NA, 4) fp16: [...,0:2]=grid*stride, [...,2:4]=anchors."""
    gat = np.zeros((128, NTSUM, NA, 4), np.float32)
    for l, L in enumerate(LEVELS):
        HW, W, stride = L["HW"], L["W"], L["stride"]
        nt = NT[l]
        hw = np.arange(nt * 128)
        gx = (hw % W).astype(np.float32) * stride
        gy = (hw // W).astype(np.float32) * stride
        gx[HW:] = 0.0
        gy[HW:] = 0.0
        sl = gat[:, LOFF[l]:LOFF[l] + nt]
        sl[:, :, :, 0] = gx.reshape(nt, 128).T[:, :, None]
        sl[:, :, :, 1] = gy.reshape(nt, 128).T[:, :, None]
        sl[:, :, :, 2:4] = np.asarray(L["anchors"], np.float32)[None, None]
    return np.ascontiguousarray(gat.astype(np.float16))


_CONSTS = None


def _make_in_maps(xs, ws, bs, use_bias):
    global _CONSTS
    if _CONSTS is None:
        _CONSTS = _host_gat()
    wxs = []
    for l, (x, w, L) in enumerate(zip(xs, ws, LEVELS)):
        HW = L["HW"]
        if L["fp8"]:
            KD = L["C"] // 256
            # [p, kd, j, 0:267] = 16*w[o, (kd*2+j)*128+p]; [.., 267:] = x
            wq = (w.T * WSCALE).astype(NP_F8).reshape(KD, 2, 128, NCOL) \
                .transpose(2, 0, 1, 3)
            xq = x.reshape(NCORES, KD, 2, 128, HW).astype(NP_F8) \
                .transpose(0, 3, 1, 2, 4)
            wx = np.zeros((NCORES, 128, KD, 2, 272 + HW), NP_F8)
            wx[:, :, :, :, 0:NCOL] = wq[None]
            wx[:, :, :, :, 272:] = xq
            wxs.append(wx)
        else:
            KC = L["C"] // 128
            wq = w.T.astype(np.float16).reshape(KC, 128, NCOL) \
                .transpose(1, 0, 2)
            xq = x.reshape(NCORES, KC, 128, HW).astype(np.float16) \
                .transpose(0, 2, 1, 3)
            wx = np.empty((NCORES, 128, KC, NCOL + HW), np.float16)
            wx[:, :, :, 0:NCOL] = wq[None]
            wx[:, :, :, NCOL:] = xq
            wxs.append(wx.reshape(NCORES, 128, KC * (NCOL + HW)))
    in_maps = []
    for core in range(NCORES):
        im = {"gat": _CONSTS}
        for l in range(len(LEVELS)):
            im[f"wx{l}"] = wxs[l][core]
            if use_bias:
                scale = WSCALE if LEVELS[l]["fp8"] else 1.0
                im[f"b{l}"] = np.ascontiguousarray(
                    (bs[l] * scale).reshape(1, NCOL).astype(np.float32))
        in_maps.append(im)
    return in_maps


def _assemble(results):
    """y85 (128,NA,nt,85) fp8 + y4 (128,NA,nt,4) fp16 -> (8, 25200, 89)."""
    out = np.empty((NCORES, 25200, NO), np.float32)
    for core in range(NCORES):
        parts = []
        for l, L in enumerate(LEVELS):
            HW = L["HW"]
            nt = NT[l]
            y85 = results[core][f"y85_{l}"].astype(np.float32)
            y4 = results[core][f"y4_{l}"].astype(np.float32)
            y = np.empty((NA, nt * 128, NO), np.float32)
            y[:, :, 4:] = y85.transpose(1, 2, 0, 3).reshape(NA, nt * 128, 85)
            y[:, :, 0:4] = y4.transpose(1, 2, 0, 3).reshape(NA, nt * 128, 4)
            parts.append(y[:, :HW, :].reshape(NA * HW, NO))
        out[core] = np.concatenate(parts, axis=0)
    return out


def _run(x0, x1, x2, w0, b0, w1, b1, w2, b2, **spmd_kwargs):
    xs = [np.asarray(x, dtype=np.float32) for x in (x0, x1, x2)]
    ws = [np.asarray(w, dtype=np.float32) for w in (w0, w1, w2)]
    bs = [np.asarray(b, dtype=np.float32) for b in (b0, b1, b2)]
    use_bias = any(np.any(b != 0) for b in bs)
    in_maps = _make_in_maps(xs, ws, bs, use_bias)
    res = run_bass_kernel_spmd(_get_prog(use_bias), in_maps,
                               core_ids=list(range(NCORES)), **spmd_kwargs)
    return _assemble(res.results), res


def kernel(x0, x1, x2, w0, b0, w1, b1, w2, b2):
    out, _ = _run(x0, x1, x2, w0, b0, w1, b1, w2, b2)
    return out


def kernel_traced(x0, x1, x2, w0, b0, w1, b1, w2, b2):
    """Like kernel() but with NTFF tracing; returns (out, BassKernelResults)."""
    return _run(x0, x1, x2, w0, b0, w1, b1, w2, b2, trace=True)
